# revision 36
# baseline (speedup 1.0000x reference)
"""Local causal (sliding-window) attention block on 8 TRN2 NeuronCores.

Reference computation (per batch b):
    h = LayerNorm(x) * gamma + beta
    Q = h@Wq, K = h@Wk, V = h@Wv          (heads: 16 x 64)
    S = QK^T/sqrt(dk) masked to causal band of width 256
    out = x + softmax(S)@V @ Wo + bo

Sharding: 8 cores = 2 batches x 4 head-groups (4 heads each).
Each core computes LN(x_b), its head-group's Q/K/V, banded attention,
and a partial out-projection  attn_g @ Wo[g]  (token-major, [T, D]).
Host reduces: out[b] = x[b] + sum_g partial[b,g] + bo.

Attention is computed key-major: for key block kb, scores
s_t[k, q] = K_kb^T Q over the query window [kb, kb+2]; exp lands P^T
directly in SBUF, band masking is one fused binary multiply against
[mka | 1 | mkb], and the AV matmul (lhsT = P^T slice, rhs = [V | 1])
produces token-major attention output with the softmax denominator as
column 64.  All four heads are interleaved in a single key-block loop:
the two heads of a pair occupy PE row-groups 0-1 / 2-3 so their score
matmuls run concurrently, and the out-projection for each finished
512-token group is emitted inline so the PE never idles long enough
for the HAM clock-gate to re-throttle it to 1.2 GHz.

gamma (and 1/sqrt(dk) for Q) are folded into the projection weights on
the host; beta enters via folded bias rows beta@W.  Weights are sent
host-side in the on-chip [P, KC, N] layout so their DMA is contiguous.
All matmul operands are bf16 (PSUM accumulation in fp32).
"""

import os

import numpy as np

import concourse.bass as bass
import concourse.tile as tile
from concourse import bacc, mybir
from concourse.bass_utils import run_bass_kernel_spmd

F32 = mybir.dt.float32
BF16 = mybir.dt.bfloat16

T = 2048          # tokens per batch
D = 1024          # model dim
HG = 4            # heads per core
DK = 64           # head dim
DG = HG * DK      # head-group feature width (256)
WIN = 256         # attention window
P = 128           # partitions
NT = T // P       # 16 token tiles
KC = D // P       # 8 feature chunks
LN_EPS = 1e-5

# filled by test.py via run(trace=True)
LAST_PROFILE = {}


def _body(tc):
    nc = tc.nc

    x = nc.dram_tensor("x", [T, D], BF16, kind="ExternalInput").ap()
    wq = nc.dram_tensor("wq", [P, KC * DG], BF16, kind="ExternalInput").ap()
    wk = nc.dram_tensor("wk", [P, KC * DG], BF16, kind="ExternalInput").ap()
    wv = nc.dram_tensor("wv", [P, KC * DG], BF16, kind="ExternalInput").ap()
    wo = nc.dram_tensor("wo", [P, (DG // P) * D], BF16, kind="ExternalInput").ap()
    bq = nc.dram_tensor("bq", [P, DG // P], F32, kind="ExternalInput").ap()
    msk3 = nc.dram_tensor("msk3", [P, 2 * 3 * P], BF16, kind="ExternalInput").ap()
    idb = nc.dram_tensor("idb", [P, P], BF16, kind="ExternalInput").ap()
    partial = nc.dram_tensor("partial", [T, D], BF16, kind="ExternalOutput").ap()

    with (
        tc.tile_pool(name="consts", bufs=1) as consts,
        tc.tile_pool(name="big", bufs=1) as big,
    ):
        # ---- resident SBUF tensors ----
        wq_sb = consts.tile([P, KC, DG], BF16, tag="wq")
        wk_sb = consts.tile([P, KC, DG], BF16, tag="wk")
        wv_sb = consts.tile([P, KC, DG], BF16, tag="wv")
        wo_sb = consts.tile([P, DG // P, D], BF16, tag="wo")
        bq_sb = consts.tile([P, DG // P], F32, tag="bq")
        msk3_sb = consts.tile([P, 2, 3 * P], BF16, tag="msk3")
        idb_sb = consts.tile([P, P], BF16, tag="idb")
        eps_sb = consts.tile([P, 1], F32, tag="eps")

        # weights first: their contiguous DMAs are what gate the first
        # projection matmuls.
        nc.sync.dma_start(out=wv_sb, in_=wv.rearrange("p (c n) -> p c n", c=KC))
        nc.sync.dma_start(out=idb_sb, in_=idb)
        nc.sync.dma_start(out=msk3_sb, in_=msk3.rearrange("p (b c) -> p b c", c=3 * P))
        nc.sync.dma_start(out=bq_sb, in_=bq)
        nc.sync.dma_start(out=wo_sb, in_=wo.rearrange("p (c n) -> p c n", c=DG // P))
        nc.vector.memset(eps_sb, LN_EPS)

        # h^T (LayerNormed x, feature-major), Q^T/K^T (feature-major),
        # V (token-major, [V | 1] per head), O^T (attn out, feature-major)
        ht_sb = big.tile([P, KC, T], BF16, tag="ht")
        qt_sb = big.tile([P, DG // P, T], BF16, tag="qt")
        kt_sb = big.tile([P, DG // P, T], BF16, tag="kt")
        v_sb = big.tile([P, NT, HG, DK + 1], BF16, tag="v")
        ot_sb = big.tile([P, DG // P, T], BF16, tag="ot")

        # ones column of [V | 1]
        nc.vector.memset(v_sb[:, :, :, DK:DK + 1], 1.0)

        # ===== Phase A+B interleaved: LayerNorm + transpose + Q/K/V =====
        # A is DVE/ACT-bound, B is PE-bound; interleaving the emission lets
        # the projection matmuls of token chunk nt backfill the PE while
        # LayerNorm of later tiles runs on the other engines.
        with (
            tc.tile_pool(name="ln", bufs=6) as ln,
            tc.tile_pool(name="lnst", bufs=9) as lnst,
            tc.tile_pool(name="abp", bufs=2, space="PSUM") as abp,
            tc.tile_pool(name="tpp", bufs=3, space="PSUM") as tpp,
        ):
            NQ = 512
            xtiles = {}
            for tb0 in range(4):
                xtl = ln.tile([P, D], BF16, tag="xt", name=f"x_{tb0}")
                nc.sync.dma_start(out=xtl, in_=x[tb0 * P:(tb0 + 1) * P, :])
                xtiles[tb0] = xtl
            nc.sync.dma_start(out=wq_sb,
                              in_=wq.rearrange("p (c n) -> p c n", c=KC))
            nc.sync.dma_start(out=wk_sb,
                              in_=wk.rearrange("p (c n) -> p c n", c=KC))
            for tb in range(NT):
                if tb in xtiles:
                    xt = xtiles.pop(tb)
                else:
                    xt = ln.tile([P, D], BF16, tag="xt", name=f"x_{tb}")
                    nc.sync.dma_start(out=xt,
                                      in_=x[tb * P:(tb + 1) * P, :])

                stats = lnst.tile([P, 2, 6], F32, tag="stats")
                xg = xt.rearrange("p (g d) -> p g d", g=2)
                nc.vector.bn_stats(out=stats[:, 0, :], in_=xg[:, 0, :])
                nc.vector.bn_stats(out=stats[:, 1, :], in_=xg[:, 1, :])
                mv = lnst.tile([P, 2], F32, tag="mv")
                nc.vector.bn_aggr(out=mv, in_=stats)

                rstd = lnst.tile([P, 1], F32, tag="rstd")
                nc.scalar.activation(
                    out=rstd, in_=mv[:, 1:2],
                    func=mybir.ActivationFunctionType.Sqrt,
                    bias=eps_sb, scale=1.0,
                )
                nc.vector.reciprocal(out=rstd, in_=rstd)
                nmr = lnst.tile([P, 1], F32, tag="nmr")
                nc.vector.scalar_tensor_tensor(
                    nmr, mv[:, 0:1], -1.0, rstd,
                    mybir.AluOpType.mult, mybir.AluOpType.mult)

                hn = ln.tile([P, D], BF16, tag="hn")
                nc.gpsimd.tensor_scalar(
                    hn[:, 0:D // 2], xt[:, 0:D // 2], rstd, nmr,
                    mybir.AluOpType.mult, mybir.AluOpType.add)
                nc.scalar.activation(
                    out=hn[:, D // 2:], in_=xt[:, D // 2:],
                    func=mybir.ActivationFunctionType.Identity,
                    bias=nmr, scale=rstd)
                for half in range(2):
                    tp = tpp.tile([P, 4 * P], BF16, tag="tp")
                    for c4 in range(4):
                        c = half * 4 + c4
                        nc.tensor.transpose(
                            tp[:, c4 * P:(c4 + 1) * P],
                            hn[:, c * P:(c + 1) * P], idb_sb)
                    dst = ht_sb[:, half * 4:(half + 1) * 4,
                                tb * P:(tb + 1) * P]
                    nc.scalar.copy(dst, tp)

                # V projection for this tile
                psv = abp.tile([P, DG], F32, tag="psv")
                for kc in range(KC):
                    nc.tensor.matmul(
                        psv,
                        ht_sb[:, kc, tb * P:(tb + 1) * P],
                        wv_sb[:, kc, :],
                        start=(kc == 0), stop=(kc == KC - 1),
                    )
                # bv is not applied on-device: softmax weights sum to 1,
                # so the V bias adds a constant bv to every attention
                # output; bv @ Wo is folded into the host-side bo add.
                vdst = v_sb[:, tb, :, 0:DK]
                vsrc = psv.rearrange("p (h d) -> p h d", h=HG)
                if tb % 2 == 0:
                    nc.vector.tensor_copy(vdst, vsrc)
                else:
                    nc.scalar.copy(vdst, vsrc)

                # Q/K projections for completed 512-token chunks 0-1;
                # chunks 2-3 are emitted inside the attention loop as
                # dense PE filler that keeps the HAM clock gate warm.
                if tb % 4 == 3 and tb < 8:
                    nt = tb // 4
                    tsl = slice(nt * NQ, (nt + 1) * NQ)
                    for oc in range(DG // P):
                        # bk is not applied: a K bias shifts all scores of
                        # a query uniformly, which softmax cancels.
                        for w_sb, dst, on_act in (
                                (wq_sb, qt_sb, True),
                                (wk_sb, kt_sb, False)):
                            ps = abp.tile([P, NQ], F32, tag="ps")
                            for kc in range(KC):
                                nc.tensor.matmul(
                                    ps,
                                    w_sb[:, kc, oc * P:(oc + 1) * P],
                                    ht_sb[:, kc, tsl],
                                    start=(kc == 0), stop=(kc == KC - 1),
                                )
                            if on_act:
                                nc.scalar.activation(
                                    out=dst[:, oc, tsl], in_=ps,
                                    func=mybir.ActivationFunctionType.Identity,
                                    bias=bq_sb[:, oc:oc + 1], scale=1.0,
                                )
                            else:
                                nc.vector.tensor_copy(dst[:, oc, tsl], ps)

        # ================= Phase C: banded attention (key-major) ==========
        # All four heads share one key-block loop.  Per kb: 4 score MMs
        # (pairs run concurrently in disjoint PE row groups), one exp and
        # one fused mask multiply per pair, then the query block kb is
        # completed for every head (up to 3 AV matmuls each), normalized
        # (denominator rides in column DK of the AV result) and
        # pair-transposed back to feature-major.  Every 4th kb the
        # finished 512-token group's out-projection is emitted inline and
        # its PSUM result is DMA'd straight to DRAM (fp32, no SBUF hop).
        with (
            tc.tile_pool(name="sp", bufs=3, space="PSUM") as sp,
            tc.tile_pool(name="avp", bufs=2, space="PSUM") as avp,
            tc.tile_pool(name="otp", bufs=1, space="PSUM") as otp,
            tc.tile_pool(name="fpp", bufs=2, space="PSUM") as fpp,
            tc.tile_pool(name="smx", bufs=8) as smx,
            tc.tile_pool(name="smst", bufs=8) as smst,
            tc.tile_pool(name="opr", bufs=4) as oprp,
            tc.tile_pool(name="fout", bufs=4) as fout,
        ):
            NO = 512
            pts = {}    # (oc, kb) -> P^T pair tile in SBUF
            otps = [None]   # current [P, 2, 4P] transpose accumulator

            def scores(kb):
                # s_t[k, q-window] for both heads of each pair; exp lands
                # P^T in SBUF; band mask (diag keeps k<=q, oldest block
                # keeps k>q) is one fused multiply against [mka | 1 | mkb]
                njb = min(3, NT - kb)
                jw = njb * P
                for oc in range(DG // P):
                    pt = smx.tile([P, 2, 3 * P], BF16, tag="pt",
                                  name=f"pt_{oc}_{kb}")
                    for hh in range(2):
                        p0 = hh * DK
                        st = sp.tile([P, 3 * P], F32, tag="st",
                                     name=f"st_{oc}_{hh}_{kb}")
                        nc.tensor.matmul(
                            st[:, :jw],
                            kt_sb[p0:p0 + DK, oc, kb * P:(kb + 1) * P],
                            qt_sb[p0:p0 + DK, oc, kb * P:kb * P + jw],
                            start=True, stop=True,
                        )
                        nc.scalar.activation(
                            out=pt[:, hh, :jw], in_=st[:, :jw],
                            func=mybir.ActivationFunctionType.Exp,
                        )
                    nc.gpsimd.tensor_mul(
                        pt[:, :, :jw], pt[:, :, :jw],
                        msk3_sb[:, :, :jw])
                    pts[(oc, kb)] = pt

            pending = []   # token tiles whose out-projection is due

            def outproj_tb(tb):
                # out-projection for one token tile -- emitted one tile
                # per key block so the dense N=512 matmuls are spread
                # evenly and the HAM clock gate never sees a low-duty
                # window
                for on in range(D // NO):
                    ps = fpp.tile([P, NO], F32, tag="fps",
                                  name=f"fps_{tb}_{on}")
                    for kd in range(DG // P):
                        nc.tensor.matmul(
                            ps,
                            ot_sb[:, kd, tb * P:(tb + 1) * P],
                            wo_sb[:, kd, on * NO:(on + 1) * NO],
                            start=(kd == 0), stop=(kd == DG // P - 1),
                        )
                    ob = fout.tile([P, NO], BF16, tag="ob")
                    if on == 0:
                        nc.vector.tensor_copy(ob, ps)
                    else:
                        nc.scalar.copy(ob, ps)
                    nc.sync.dma_start(
                        out=partial[tb * P:(tb + 1) * P,
                                    on * NO:(on + 1) * NO],
                        in_=ob)

            def consume(kb):
                # query block kb is complete in every head's P^T tiles:
                # AV (token-major, denominator in column DK) for all four
                # heads into one packed PSUM tile, then normalize and
                # transpose both heads of a pair in one [128,128] shot.
                av = avp.tile([P, HG, 2 * DK], F32, tag="av",
                              name=f"av_{kb}")
                kbs = list(range(max(kb - 2, 0), kb + 1))
                for oc in range(DG // P):
                    for hh in range(2):
                        h = oc * 2 + hh
                        for i, kb2 in enumerate(kbs):
                            qoff = kb - kb2
                            nc.tensor.matmul(
                                av[:, h, 0:DK + 1],
                                pts[(oc, kb2)][:, hh,
                                               qoff * P:(qoff + 1) * P],
                                v_sb[:, kb2, h, :],
                                start=(i == 0), stop=(kb2 == kb),
                            )
                if kb % 4 == 0:
                    otps[0] = otp.tile([P, 2, 4 * P], BF16, tag="otb",
                                       name=f"otb_{kb}")
                for oc in range(DG // P):
                    op = oprp.tile([P, 2, DK], BF16, tag="opr",
                                   name=f"opr_{oc}_{kb}")
                    for hh in range(2):
                        h = oc * 2 + hh
                        rden = smst.tile([P, 1], F32, tag="rden")
                        nc.vector.reciprocal(out=rden,
                                             in_=av[:, h, DK:DK + 1])
                        if h % 2 == 0:
                            nc.vector.tensor_scalar_mul(
                                op[:, hh, :], av[:, h, 0:DK], rden)
                        else:
                            nc.scalar.activation(
                                out=op[:, hh, :], in_=av[:, h, 0:DK],
                                func=mybir.ActivationFunctionType.Identity,
                                scale=rden)
                    nc.tensor.transpose(
                        otps[0][:, oc, (kb % 4) * P:(kb % 4 + 1) * P],
                        op.rearrange("p a b -> p (a b)"), idb_sb)
                    if kb % 4 == 3:
                        g = kb // 4
                        dst = ot_sb[:, oc, g * 4 * P:(g + 1) * 4 * P]
                        nc.vector.tensor_copy(dst, otps[0][:, oc, :])
                        if oc == 1:
                            pending.extend(range(4 * g, 4 * g + 4))
                # retire P^T tiles no longer referenced
                if kb - 2 >= 0:
                    for oc in range(DG // P):
                        pts.pop((oc, kb - 2), None)

            def qk_set(nt, si):
                # one (oc, proj) Q/K projection set of chunk nt, using
                # the outproj PSUM class as scratch
                tsl = slice(nt * NO, (nt + 1) * NO)
                oc, pi = si // 2, si % 2
                w_sb, dst, on_act = ((wq_sb, qt_sb, True),
                                     (wk_sb, kt_sb, False))[pi]
                ps = fpp.tile([P, NO], F32, tag="fps",
                              name=f"qk_{nt}_{si}")
                for kc in range(KC):
                    nc.tensor.matmul(
                        ps,
                        w_sb[:, kc, oc * P:(oc + 1) * P],
                        ht_sb[:, kc, tsl],
                        start=(kc == 0), stop=(kc == KC - 1),
                    )
                if on_act:
                    nc.vector.tensor_scalar_add(
                        dst[:, oc, tsl], ps, bq_sb[:, oc:oc + 1])
                else:
                    nc.vector.tensor_copy(dst[:, oc, tsl], ps)

            # 1-deep software pipeline: scores(kb+1) is emitted before
            # the AV of kb so the PE always has independent matmuls ready
            # while exp/mask complete.  The first 8 key blocks each carry
            # one Q/K projection set of chunks 2/3 as low-priority dense
            # filler (their readers scores(8)/scores(12) come later).
            scores(0)
            for kb in range(NT):
                if kb + 1 < NT:
                    scores(kb + 1)
                consume(kb)
                if pending:
                    outproj_tb(pending.pop(0))
                if kb < 8:
                    qk_set(2 + kb // 4, kb % 4)
            while pending:
                outproj_tb(pending.pop(0))


def build_nc():
    nc = bacc.Bacc("TRN2", target_bir_lowering=False, debug=False,
                   num_devices=8)
    with tile.TileContext(nc) as tc:
        _body(tc)
    nc.compile()
    return nc


def _prep_core_inputs(x, Wq, Wk, Wv, Wo, gamma, beta):
    """Host-side prep: per-(batch, head-group) input dicts."""
    import ml_dtypes
    BF = ml_dtypes.bfloat16
    B = x.shape[0]
    ii = np.arange(P)[:, None]   # key index within block (rows)
    jj = np.arange(P)[None, :]   # query index within block (cols)
    # fused band mask [mka | 1 | mkb] per head of a pair: diag block
    # keeps k <= q, middle block is fully valid, oldest keeps k > q.
    mka = (ii <= jj).astype(np.float32)
    ones = np.ones((P, P), np.float32)
    mkb = (ii > jj).astype(np.float32)
    m3 = np.concatenate([mka, ones, mkb], axis=1)
    msk3_np = np.concatenate([m3, m3], axis=1).astype(BF)
    idb_np = np.eye(P, dtype=np.float32).astype(BF)

    def onchip(w, kc):
        # [kc*P, N] -> [P, kc*N] so the weight DMA is contiguous
        n = w.shape[1]
        return np.ascontiguousarray(
            w.reshape(kc, P, n).transpose(1, 0, 2).reshape(P, kc * n))

    in_maps = []
    for b in range(B):
        for g in range(4):
            sl = slice(g * DG, (g + 1) * DG)
            sq = np.float32(1.0 / np.sqrt(DK))
            wq_g = (gamma[:, None] * Wq[:, sl] * sq).astype(BF)
            wk_g = (gamma[:, None] * Wk[:, sl]).astype(BF)
            wv_g = (gamma[:, None] * Wv[:, sl]).astype(BF)
            bq_g = ((beta @ Wq[:, sl]) * sq).astype(np.float32)
            in_maps.append({
                "x": np.ascontiguousarray(x[b]).astype(BF),
                "wq": onchip(wq_g, KC), "wk": onchip(wk_g, KC),
                "wv": onchip(wv_g, KC),
                "wo": onchip(Wo[sl, :].astype(BF), DG // P),
                "bq": np.ascontiguousarray(bq_g.reshape(DG // P, P).T),
                "msk3": msk3_np, "idb": idb_np,
            })
    return in_maps


def _ntff_hook(so_path="/opt/axon/libaxon_pjrt.so"):
    import contextlib
    import ctypes

    lib = ctypes.CDLL(so_path)
    lib.axon_start_nrt_profile.argtypes = [
        ctypes.POINTER(ctypes.c_int64), ctypes.c_size_t]
    lib.axon_start_nrt_profile.restype = ctypes.c_int64
    lib.axon_stop_nrt_profile.argtypes = [ctypes.c_char_p]
    lib.axon_stop_nrt_profile.restype = ctypes.c_int64

    @contextlib.contextmanager
    def _hook(output_dir, device_ids):
        import jax
        jax.devices()
        if device_ids:
            ids = (ctypes.c_int64 * len(device_ids))(*device_ids)
            rc = lib.axon_start_nrt_profile(ids, len(device_ids))
        else:
            rc = lib.axon_start_nrt_profile(None, 0)
        if rc != 0:
            raise RuntimeError(f"axon_start_nrt_profile rc={rc}")
        try:
            yield
        finally:
            n = lib.axon_stop_nrt_profile(str(output_dir).encode())
            print(f"profile: {n} file(s) written to {output_dir}")

    return _hook


def _run_traced(nc, in_maps, trace_dir=None):
    """Execute via PJRT with NTFF capture; return BassKernelResults with
    exec_time_ns and a perfetto trace."""
    import glob
    import tempfile

    import gauge.profiler
    from concourse import bass2jax, bass_utils
    from concourse._compat import FishPath

    neff_dir = trace_dir or tempfile.mkdtemp(prefix="trn_trace_")
    hook = _ntff_hook()
    with hook(neff_dir, [0]):
        results = bass2jax.run_bass_via_pjrt(nc, in_maps, n_cores=len(in_maps))

    ntffs = glob.glob(os.path.join(neff_dir, "*_body*.ntff"))
    if not ntffs:
        print(f"no ntffs in {neff_dir}: {os.listdir(neff_dir)}")
        return bass_utils.BassKernelResults(
            results=results, instructions_and_trace=None,
            profile_json=None, exec_time_ns=None)

    profile = gauge.profiler.Profile(
        profile_path=FishPath(neff_dir),
        kernel_dev_mode=True,
        profile_on_exit=False,
        bass_kernel=nc.m,
        offline_processing=True,
        fname="*_body*",
        metadata={},
    )
    return bass_utils._process_ntff_profile(
        profile, neff_dir, nc, list(range(len(in_maps))),
        None, False, {}, trace_events=False,
    ).as_bass_kernel_results(results)


def kernel(x, Wq, Wk, Wv, Wo, bo, gamma, beta, trace=False):
    global LAST_PROFILE
    x = np.asarray(x, dtype=np.float32)
    Wq, Wk, Wv, Wo = (np.asarray(a, dtype=np.float32) for a in (Wq, Wk, Wv, Wo))
    bo = np.asarray(bo, dtype=np.float32)
    gamma = np.asarray(gamma, dtype=np.float32)
    beta = np.asarray(beta, dtype=np.float32)

    nc = build_nc()
    in_maps = _prep_core_inputs(x, Wq, Wk, Wv, Wo, gamma, beta)
    if trace:
        res = _run_traced(nc, in_maps)
    else:
        res = run_bass_kernel_spmd(nc, in_maps, core_ids=list(range(8)))
    LAST_PROFILE = {"exec_time_ns": res.exec_time_ns}

    # the V bias is not applied on-device: softmax rows sum to 1, so it
    # contributes the constant (beta @ Wv) @ Wo to every token.
    bv_full = (beta @ Wv).astype(np.float32)
    const_row = bo + bv_full @ Wo

    B = x.shape[0]
    out = np.empty_like(x)
    for b in range(B):
        acc = x[b] + const_row[None, :]
        for g in range(4):
            acc = acc + res.results[b * 4 + g]["partial"].astype(np.float32)
        out[b] = acc
    return out
